# revision 2
# baseline (speedup 1.0000x reference)
"""Trainium2 Bass kernel for a local-window multi-head attention block.

Math (per batch element b, all in one NeuronCore; batch is data-parallel
across the 8 cores):
    qkv  = x @ w_qkv.T                      [N, 2304]
    q,k,v split into 12 heads of dim 64, q scaled by 1/8
    S    = q @ k.T + local mask             (mask: |dh|<=3, |dw|<=5 on a 16x64 grid)
    P    = softmax(S); O = P @ v
    out  = O @ w_proj.T + b_proj

Device layout notes:
  - Tokens are permuted on the host to w-major order (n' = 16*w + h).
    The local window |dw|<=5 then spans only 11 of 64 w-columns, so each
    128-key chunk's visible queries fit in a 288-wide contiguous window
    (vs 512 in h-major order): S/PV matmuls, exp and mask work all
    shrink by ~45%.
  - Everything is computed transposed (channels on partitions):
    qkvT = w_qkv @ x.T via PE, S^T tiles per 128-key chunk over a
    288-wide query window, softmax without max-subtraction (scores are
    tiny), row sums via an appended ones-column in the P@V matmul, 1/r
    via a fast DVE reciprocal and a GPSIMD partition broadcast.
  - q/k/v matmuls run in bf16, the output projection in float32r.
    Weights are laid out on the host so every DMA is contiguous per
    partition.
  - Program order interleaves per-pair q/k projection with the previous
    pair's attention so the PE fills the gaps of the ACT-bound softmax.
"""

import os
import sys

sys.path.insert(0, "/opt/trn_rl_repo")

import numpy as np

B, N, DIM = 8, 1024, 768
NH, HD = 12, 64
SCALE = HD ** -0.5
P = 128
CN = DIM // P            # 6 channel chunks
MC = N // P              # 8 token chunks
TWIN = 512               # output/projection tile width (1 psum bank)
NT = N // TWIN           # 2 output tiles
AWIN = 288               # attention query window per 128-key chunk (w-major)
WIN_START = [min(max(128 * c - 80, 0), N - AWIN) for c in range(MC)]

# w-major permutation: new token n' = 16*w + h  ->  original n = 64*h + w
PERM = np.array([64 * (i % 16) + (i // 16) for i in range(N)])

# host weight layout offsets (all contiguous per partition):
#   [p, hp, qk, co, 128] for q/k: 6 pairs * 2 * 6 * 128 = 9216
#   [p, oh, co, 384] for v: 2 * 6 * 384 = 4608
WQK_SZ = 2 * CN * P      # 1536 per (hp)
WV_OFF = CN * WQK_SZ     # 9216
WQKV_COLS = WV_OFF + 2 * CN * 384  # 13824

_PROG = None


def _emit(ctx, tc, aps, debug=None):
    import concourse.bass as bass
    import concourse.mybir as mybir

    nc = tc.nc
    f32 = mybir.dt.float32
    bf16 = mybir.dt.bfloat16
    f32r = mybir.dt.float32r
    AF = mybir.ActivationFunctionType

    xT, wqkvT, wprojT, biasT, bmask, outT = aps

    consts = ctx.enter_context(tc.tile_pool(name="consts", bufs=1))
    wpool = ctx.enter_context(tc.tile_pool(name="wstream", bufs=4))
    psum = ctx.enter_context(tc.tile_pool(name="ps", bufs=2, space="PSUM"))
    etpool = ctx.enter_context(tc.tile_pool(name="etp", bufs=4))
    rpool = ctx.enter_context(tc.tile_pool(name="rp", bufs=4))
    bpool = ctx.enter_context(tc.tile_pool(name="invbp", bufs=3))
    stpool = ctx.enter_context(tc.tile_pool(name="stp", bufs=2))
    expool = ctx.enter_context(tc.tile_pool(name="exp_scratch", bufs=3))

    def load_wqk(hp):
        """Fetch the q and k weight chunks for head pair hp."""
        tiles = []
        for qk in range(2):
            base = (2 * hp + qk) * CN * P
            w = wpool.tile([P, CN, P], bf16, name="w%d" % qk, tag="wqk")
            nc.sync.dma_start(
                w[:], wqkvT[:, base: base + CN * P].rearrange(
                    "p (c o) -> p c o", c=CN))
            tiles.append(w)
        return tiles

    # Inputs needed first: x chunk 0 + pair-0 q/k weights to start the PE.
    xT_r = xT.rearrange("p (co n) -> p co n", co=CN)
    xT_sb = consts.tile([P, CN, N], bf16)
    nc.sync.dma_start(xT_sb[:, 0, :], xT_r[:, 0, :])
    wqk0 = load_wqk(0)
    for co in range(1, CN):
        nc.sync.dma_start(xT_sb[:, co, :], xT_r[:, co, :])
    wv0 = wpool.tile([P, CN, 384], bf16, name="wv0", tag="wv")
    nc.sync.dma_start(
        wv0[:], wqkvT[:, WV_OFF: WV_OFF + CN * 384].rearrange(
            "p (c o) -> p c o", c=CN))
    wv1 = wpool.tile([P, CN, 384], bf16, name="wv1", tag="wv")
    nc.sync.dma_start(
        wv1[:], wqkvT[:, WV_OFF + CN * 384:].rearrange(
            "p (c o) -> p c o", c=CN))
    # proj weights early: they only need DMA bandwidth, and loading them
    # here removes the PE stall before the output projection.
    wprojT_sb = consts.tile([P, CN, DIM], f32r)
    nc.sync.dma_start(
        wprojT_sb[:], wprojT.rearrange("p (c o) -> p c o", c=CN))
    bias_sb = consts.tile([P, CN], f32)
    nc.sync.dma_start(bias_sb[:], biasT[:])
    bmask_sb = consts.tile([P, MC, AWIN], bf16)
    nc.sync.dma_start(bmask_sb[:], bmask[:])

    qkT_sb = consts.tile([P, 2 * CN, N], bf16)     # chunks 0..5 = q, 6..11 = k
    V_sb = consts.tile([P, MC, NH + 1, 66], bf16)  # col 64 = ones; ghost
    # head slot at the end so a 128-wide lhsT slice never leaves the tensor
    V_flat = V_sb.rearrange("p m h c -> p m (h c)")
    OT_sb = consts.tile([P, CN, N], f32r)
    nc.gpsimd.memset(V_sb[:], 0.0)
    nc.vector.memset(V_sb[:, :, :, 64:65], 1.0)
    if debug is not None:
        debug.update(qkT_sb=qkT_sb, V_sb=V_sb, OT_sb=OT_sb)

    def emit_qk(hp, tiles):
        """q/k projection for head pair hp: psum [o-chunk, n] transposed."""
        for qk in range(2):
            w = tiles[qk]
            for t in range(NT):
                qps = psum.tile([P, TWIN], f32, name="qps", tag="qkv")
                for k in range(CN):
                    nc.tensor.matmul(
                        qps[:],
                        lhsT=w[:, k, :],
                        rhs=xT_sb[:, k, TWIN * t: TWIN * (t + 1)],
                        start=(k == 0),
                        stop=(k == CN - 1),
                    )
                nc.vector.tensor_copy(
                    qkT_sb[:, CN * qk + hp, TWIN * t: TWIN * (t + 1)], qps[:])

    def emit_v():
        """v projection: psum [n-chunk, o], n on partitions."""
        for oh, wv in ((0, wv0), (1, wv1)):
            for m in range(MC):
                vps = psum.tile([P, TWIN], f32, name="vps", tag="qkv")
                for k in range(CN):
                    nc.tensor.matmul(
                        vps[:, 0:384],
                        lhsT=xT_sb[:, k, P * m: P * (m + 1)],
                        rhs=wv[:, k, :],
                        start=(k == 0),
                        stop=(k == CN - 1),
                    )
                nc.scalar.activation(
                    V_sb[:, m, 6 * oh: 6 * (oh + 1), 0:64],
                    vps[:, 0:384].rearrange("p (a b) -> p a b", b=64),
                    AF.Copy,
                )

    def emit_attention(hp):
        """S, softmax and P@V for head pair hp (both heads interleaved)."""
        ets = [etpool.tile([P, MC, AWIN], bf16, name="et%d" % hh, tag="et")
               for hh in range(2)]
        for cp in range(MC // 2):
            spss = [psum.tile([P, 2 * TWIN], f32, name="sps%d" % hh, tag="sps")
                    for hh in range(2)]
            for j in range(2):
                c = 2 * cp + j
                s = WIN_START[c]
                for hh in range(2):
                    prange = slice(64 * hh, 64 * hh + 64)
                    nc.tensor.matmul(
                        spss[hh][:, TWIN * j: TWIN * j + AWIN],
                        lhsT=qkT_sb[prange, CN + hp, P * c: P * (c + 1)],
                        rhs=qkT_sb[prange, hp, s: s + AWIN],
                        start=True,
                        stop=True,
                        tile_position=(64 * hh, 0),
                    )
            for hh in range(2):
                esc = expool.tile([P, 2, AWIN], bf16, name="esc", tag="esc")
                nc.scalar.activation(
                    esc[:],
                    spss[hh][:].rearrange("p (j w) -> p j w", j=2)[:, :, 0:AWIN],
                    AF.Exp,
                )
                nc.vector.tensor_mul(
                    ets[hh][:, 2 * cp: 2 * cp + 2, :].rearrange("p a b -> p (a b)"),
                    esc[:].rearrange("p a b -> p (a b)"),
                    bmask_sb[:, 2 * cp: 2 * cp + 2, :].rearrange("p a b -> p (a b)"),
                )
        for hh in range(2):
            h = 2 * hp + hh
            et = ets[hh]
            if debug is not None and ("d_et%d" % h) in debug:
                nc.sync.dma_start(debug["d_et%d" % h][:], et[:])
            # P@V with ones column: psum rows 0:64 = O^T, row 64 = sum(E).
            for t in range(NT):
                ot = psum.tile([P, TWIN], f32, name="ot", tag="ot")
                cs = [c for c in range(MC)
                      if min(WIN_START[c] + AWIN, TWIN * (t + 1)) > max(WIN_START[c], TWIN * t)]
                # widest-overlap chunk first so the start=True matmul covers
                # the largest psum range (per-element has_written then only
                # ever accumulates into written elements)
                cs.sort(key=lambda c: max(WIN_START[c], TWIN * t)
                        - min(WIN_START[c] + AWIN, TWIN * (t + 1)))
                for i, c in enumerate(cs):
                    lo = max(WIN_START[c], TWIN * t)
                    hi = min(WIN_START[c] + AWIN, TWIN * (t + 1))
                    # 128-wide weight load (vs 65) keeps FWL enabled; psum
                    # rows 65:128 receive garbage and are never read
                    nc.tensor.matmul(
                        ot[:, lo - TWIN * t: hi - TWIN * t],
                        lhsT=V_flat[:, c, 66 * h: 66 * h + 128],
                        rhs=et[:, c, lo - WIN_START[c]: hi - WIN_START[c]],
                        start=(i == 0),
                        stop=(i == len(cs) - 1),
                    )
                # reciprocal_approx_fast misreads PSUM operands on HW;
                # stage the r-row through SBUF first
                rrow = rpool.tile([1, TWIN], f32, name="rrow", tag="rrow")
                nc.scalar.activation(rrow[:], ot[64:65, :], AF.Copy)
                invrow = rpool.tile([1, TWIN], f32, name="invrow", tag="invrow")
                nc.vector.reciprocal_approx_fast(invrow[:], rrow[:])
                invb = bpool.tile([64, TWIN], f32, name="invb", tag="invb")
                nc.gpsimd.partition_broadcast(invb[:], invrow[:])
                if hh == 0:
                    nc.vector.tensor_mul(
                        OT_sb[0:64, hp, TWIN * t: TWIN * (t + 1)],
                        ot[0:64, :], invb[:],
                    )
                else:
                    st = stpool.tile([64, TWIN], f32r, name="st", tag="st")
                    nc.vector.tensor_mul(st[:], ot[0:64, :], invb[:])
                    nc.sync.dma_start(
                        OT_sb[64:128, hp, TWIN * t: TWIN * (t + 1)], st[:])

    # pair-0 q/k first (starts the PE earliest), then v, then interleave:
    # attention for pair hp overlaps the qk projection for pair hp+1.
    emit_qk(0, wqk0)
    emit_v()
    wq_next = load_wqk(1)
    for hp in range(CN):
        wq_cur = wq_next
        if hp + 1 < CN:
            wq_next = load_wqk(hp + 2) if hp + 2 < CN else None
            emit_qk(hp + 1, wq_cur)
        emit_attention(hp)

    # ---------------- output projection ----------------
    with tc.tile_pool(name="outst", bufs=4) as ostpool:
        for oc in range(CN):
            for t in range(NT):
                pps = psum.tile([P, TWIN], f32, name="pps", tag="qkv")
                for k in range(CN):
                    nc.tensor.matmul(
                        pps[:],
                        lhsT=wprojT_sb[:, k, P * oc: P * (oc + 1)],
                        rhs=OT_sb[:, k, TWIN * t: TWIN * (t + 1)],
                        start=(k == 0),
                        stop=(k == CN - 1),
                    )
                ost = ostpool.tile([P, TWIN], f32, name="ost", tag="ost")
                nc.vector.tensor_add(
                    ost[:], pps[:],
                    bias_sb[:, oc: oc + 1].to_broadcast((P, TWIN)))
                nc.sync.dma_start(
                    outT[P * oc: P * (oc + 1), TWIN * t: TWIN * (t + 1)], ost[:])


def _build(debug_shapes=False):
    global _PROG
    if _PROG is not None:
        return _PROG
    from contextlib import ExitStack

    from concourse import bacc
    import concourse.mybir as mybir
    import concourse.tile as tile

    f32 = mybir.dt.float32
    bf16 = mybir.dt.bfloat16
    f32r = mybir.dt.float32r

    nc = bacc.Bacc("TRN2", target_bir_lowering=False, debug=False,
                   enable_asserts=False)
    xT = nc.dram_tensor("xT", [P, CN * N], bf16, kind="ExternalInput").ap()
    wqkvT = nc.dram_tensor("wqkvT", [P, WQKV_COLS], bf16, kind="ExternalInput").ap()
    wprojT = nc.dram_tensor("wprojT", [P, CN * DIM], f32r, kind="ExternalInput").ap()
    biasT = nc.dram_tensor("biasT", [P, CN], f32, kind="ExternalInput").ap()
    bmask = nc.dram_tensor("bmask", [P, MC, AWIN], bf16, kind="ExternalInput").ap()
    outT = nc.dram_tensor("outT", [DIM, N], f32, kind="ExternalOutput").ap()

    with tile.TileContext(nc) as tc:
        with ExitStack() as ctx:
            _emit(ctx, tc, (xT, wqkvT, wprojT, biasT, bmask, outT))
    nc.compile()
    _PROG = nc
    return nc


def _host_inputs(x, w_qkv, w_proj, b_proj, mask):
    import ml_dtypes

    x = np.asarray(x, dtype=np.float32)
    w_qkv = np.asarray(w_qkv, dtype=np.float32)
    w_proj = np.asarray(w_proj, dtype=np.float32)
    b_proj = np.asarray(b_proj, dtype=np.float32)
    mask = np.asarray(mask, dtype=np.float32)

    wq = w_qkv.copy()
    wq[0:DIM] *= SCALE
    wT = np.ascontiguousarray(wq.T)                          # [768 in, 2304 out]
    # q/k blocks: [p, hp, qk, co, 128] ; v blocks: [p, oh, co, 384]
    wqkv_host = np.empty((P, WQKV_COLS), dtype=np.float32)
    for hp in range(CN):
        for qk in range(2):
            blk = wT[:, DIM * qk + P * hp: DIM * qk + P * hp + P]  # [768, 128]
            blk = blk.reshape(CN, P, P).transpose(1, 0, 2).reshape(P, CN * P)
            base = (2 * hp + qk) * CN * P
            wqkv_host[:, base: base + CN * P] = blk
    for oh in range(2):
        blk = wT[:, 2 * DIM + 384 * oh: 2 * DIM + 384 * (oh + 1)]  # [768, 384]
        blk = blk.reshape(CN, P, 384).transpose(1, 0, 2).reshape(P, CN * 384)
        wqkv_host[:, WV_OFF + oh * CN * 384: WV_OFF + (oh + 1) * CN * 384] = blk
    wqkvT = wqkv_host.astype(ml_dtypes.bfloat16)

    wprojT = np.ascontiguousarray(
        w_proj.T.reshape(CN, P, DIM).transpose(1, 0, 2).reshape(P, CN * DIM))
    biasT = np.ascontiguousarray(b_proj.reshape(CN, P).T)    # [128, 6]

    vis = (mask[0, 0] == 0.0)
    vis_w = vis[np.ix_(PERM, PERM)]
    bm = np.zeros((P, MC, AWIN), dtype=ml_dtypes.bfloat16)
    for c in range(MC):
        s = WIN_START[c]
        bm[:, c, :] = vis_w[c * P:(c + 1) * P, s: s + AWIN]

    in_maps = []
    for b in range(B):
        xTb = np.ascontiguousarray(x[b].T[:, PERM])          # [768, 1024] w-major
        xTb = xTb.reshape(CN, P, N).transpose(1, 0, 2).reshape(P, CN * N)
        in_maps.append({
            "xT": xTb.astype(ml_dtypes.bfloat16),
            "wqkvT": wqkvT,
            "wprojT": wprojT,
            "biasT": biasT,
            "bmask": bm,
        })
    return in_maps


PROFILE = False
LAST_RESULT = None


def kernel(x, w_qkv, w_proj, b_proj, mask):
    global LAST_RESULT
    from concourse.bass_utils import run_bass_kernel_spmd

    nc = _build()
    in_maps = _host_inputs(x, w_qkv, w_proj, b_proj, mask)
    res = run_bass_kernel_spmd(nc, in_maps, core_ids=list(range(B)),
                               trace=PROFILE)
    LAST_RESULT = res
    out = np.empty((B, N, DIM), dtype=np.float32)
    for b in range(B):
        out[b][PERM, :] = np.asarray(res.results[b]["outT"]).T
    return np.ascontiguousarray(out)


# revision 8
# speedup vs baseline: 1.1781x; 1.1781x over previous
"""Trainium2 Bass kernel for a local-window multi-head attention block.

Math (per batch element b, all in one NeuronCore; batch is data-parallel
across the 8 cores):
    qkv  = x @ w_qkv.T                      [N, 2304]
    q,k,v split into 12 heads of dim 64, q scaled by 1/8
    S    = q @ k.T + local mask             (mask: |dh|<=3, |dw|<=5 on a 16x64 grid)
    P    = softmax(S); O = P @ v
    out  = O @ w_proj.T + b_proj

Device layout notes:
  - Tokens are permuted on the host to w-major order (n' = 16*w + h).
    The local window |dw|<=5 then spans only 11 of 64 w-columns, so each
    128-key chunk's visible queries fit in a 288-wide contiguous window
    (vs 512 in h-major order): S/PV matmuls, exp and mask work all
    shrink by ~45%.
  - Everything is computed transposed (channels on partitions):
    qkvT = w_qkv @ x.T via PE, S^T tiles per 128-key chunk over a
    288-wide query window, softmax without max-subtraction (scores are
    tiny), row sums via an appended ones-column in the P@V matmul, 1/r
    via a fast DVE reciprocal and a GPSIMD partition broadcast.
  - q/k/v matmuls run in bf16, the output projection in float32r.
    Weights are laid out on the host so every DMA is contiguous per
    partition.
  - Program order interleaves per-pair q/k projection with the previous
    pair's attention so the PE fills the gaps of the ACT-bound softmax.
"""

import os
import sys

sys.path.insert(0, "/opt/trn_rl_repo")

import numpy as np

B, N, DIM = 8, 1024, 768
NH, HD = 12, 64
SCALE = HD ** -0.5
P = 128
CN = DIM // P            # 6 channel chunks
MC = N // P              # 8 token chunks
TWIN = 512               # output/projection tile width (1 psum bank)
NT = N // TWIN           # 2 output tiles
AWIN = 288               # attention query window per 128-key chunk (w-major)
WIN_START = [min(max(128 * c - 80, 0), N - AWIN) for c in range(MC)]

# w-major permutation: new token n' = 16*w + h  ->  original n = 64*h + w
PERM = np.array([64 * (i % 16) + (i // 16) for i in range(N)])

# host weight layout offsets (all contiguous per partition):
#   [p, hp, qk, co, 128] for q/k: 6 pairs * 2 * 6 * 128 = 9216
#   [p, oh, co, 384] for v: 2 * 6 * 384 = 4608
WQK_SZ = 2 * CN * P      # 1536 per (hp)
WV_OFF = CN * WQK_SZ     # 9216
WQKV_COLS = WV_OFF + 2 * CN * 384  # 13824

_PROG = None


def _emit(ctx, tc, aps, debug=None):
    import concourse.bass as bass
    import concourse.mybir as mybir

    nc = tc.nc
    f32 = mybir.dt.float32
    bf16 = mybir.dt.bfloat16
    f32r = mybir.dt.float32r
    AF = mybir.ActivationFunctionType

    xT, wqkvT, wprojT, biasT, bmask, outT = aps

    consts = ctx.enter_context(tc.tile_pool(name="consts", bufs=1))
    wpool = ctx.enter_context(tc.tile_pool(name="wstream", bufs=4))
    psum = ctx.enter_context(tc.tile_pool(name="ps", bufs=2, space="PSUM"))
    etpool = ctx.enter_context(tc.tile_pool(name="etp", bufs=4))
    rpool = ctx.enter_context(tc.tile_pool(name="rp", bufs=4))
    bpool = ctx.enter_context(tc.tile_pool(name="invbp", bufs=3))
    stpool = ctx.enter_context(tc.tile_pool(name="stp", bufs=2))
    expool = ctx.enter_context(tc.tile_pool(name="exp_scratch", bufs=3))

    def load_wqk(hp):
        """Fetch the q and k weight chunks for head pair hp."""
        tiles = []
        for qk in range(2):
            base = (2 * hp + qk) * CN * P
            w = wpool.tile([P, CN, P], bf16, name="w%d" % qk, tag="wqk")
            nc.sync.dma_start(
                w[:], wqkvT[:, base: base + CN * P].rearrange(
                    "p (c o) -> p c o", c=CN))
            tiles.append(w)
        return tiles

    # Inputs needed first: x chunk 0 + pair-0 q/k weights to start the PE.
    xT_r = xT.rearrange("p (co n) -> p co n", co=CN)
    xT_sb = consts.tile([P, CN, N], bf16)
    nc.sync.dma_start(xT_sb[:, 0, :], xT_r[:, 0, :])
    wqk0 = load_wqk(0)
    for co in range(1, CN):
        nc.sync.dma_start(xT_sb[:, co, :], xT_r[:, co, :])
    wv0 = wpool.tile([P, CN, 384], bf16, name="wv0", tag="wv")
    nc.sync.dma_start(
        wv0[:], wqkvT[:, WV_OFF: WV_OFF + CN * 384].rearrange(
            "p (c o) -> p c o", c=CN))
    wv1 = wpool.tile([P, CN, 384], bf16, name="wv1", tag="wv")
    nc.sync.dma_start(
        wv1[:], wqkvT[:, WV_OFF + CN * 384:].rearrange(
            "p (c o) -> p c o", c=CN))
    # proj weights early: they only need DMA bandwidth, and loading them
    # here removes the PE stall before the output projection.
    wprojT_sb = consts.tile([P, CN, DIM], f32r)
    nc.sync.dma_start(
        wprojT_sb[:], wprojT.rearrange("p (c o) -> p c o", c=CN))
    bias_sb = consts.tile([P, CN], f32)
    nc.sync.dma_start(bias_sb[:], biasT[:])
    bmask_sb = consts.tile([P, MC, AWIN], bf16)
    nc.sync.dma_start(bmask_sb[:], bmask[:])

    qkT_sb = consts.tile([P, 2 * CN, N], bf16)     # chunks 0..5 = q, 6..11 = k
    V_sb = consts.tile([P, MC, NH + 1, 66], bf16)  # col 64 = ones; ghost
    # head slot at the end so a 128-wide lhsT slice never leaves the tensor
    V_flat = V_sb.rearrange("p m h c -> p m (h c)")
    OT_sb = consts.tile([P, CN, N], f32r)
    nc.gpsimd.memset(V_sb[:], 0.0)
    nc.vector.memset(V_sb[:, :, :, 64:65], 1.0)
    if debug is not None:
        debug.update(qkT_sb=qkT_sb, V_sb=V_sb, OT_sb=OT_sb)

    def emit_qk(hp, tiles):
        """q/k projection for head pair hp: psum [o-chunk, n] transposed."""
        for qk in range(2):
            w = tiles[qk]
            for t in range(NT):
                qps = psum.tile([P, TWIN], f32, name="qps", tag="qkv")
                for k in range(CN):
                    nc.tensor.matmul(
                        qps[:],
                        lhsT=w[:, k, :],
                        rhs=xT_sb[:, k, TWIN * t: TWIN * (t + 1)],
                        start=(k == 0),
                        stop=(k == CN - 1),
                    )
                nc.vector.tensor_copy(
                    qkT_sb[:, CN * qk + hp, TWIN * t: TWIN * (t + 1)], qps[:])

    def emit_v(oh, ms):
        """v projection: psum [n-chunk, o], n on partitions."""
        wv = wv0 if oh == 0 else wv1
        for m in ms:
            vps = psum.tile([P, TWIN], f32, name="vps", tag="qkv")
            for k in range(CN):
                nc.tensor.matmul(
                    vps[:, 0:384],
                    lhsT=xT_sb[:, k, P * m: P * (m + 1)],
                    rhs=wv[:, k, :],
                    start=(k == 0),
                    stop=(k == CN - 1),
                )
            nc.scalar.activation(
                V_sb[:, m, 6 * oh: 6 * (oh + 1), 0:64],
                vps[:, 0:384].rearrange("p (a b) -> p a b", b=64),
                AF.Copy,
            )

    def emit_attention(hp):
        """S, softmax and P@V for head pair hp (both heads interleaved)."""
        ets = [etpool.tile([P, MC, AWIN], bf16, name="et%d" % hh, tag="et")
               for hh in range(2)]
        for cp in range(MC // 2):
            spss = [psum.tile([P, 2 * TWIN], f32, name="sps%d" % hh, tag="sps")
                    for hh in range(2)]
            for j in range(2):
                c = 2 * cp + j
                s = WIN_START[c]
                for hh in range(2):
                    prange = slice(64 * hh, 64 * hh + 64)
                    nc.tensor.matmul(
                        spss[hh][:, TWIN * j: TWIN * j + AWIN],
                        lhsT=qkT_sb[prange, CN + hp, P * c: P * (c + 1)],
                        rhs=qkT_sb[prange, hp, s: s + AWIN],
                        start=True,
                        stop=True,
                        tile_position=(64 * hh, 0),
                    )
            for hh in range(2):
                esc = expool.tile([P, 2, AWIN], bf16, name="esc", tag="esc")
                nc.scalar.activation(
                    esc[:],
                    spss[hh][:].rearrange("p (j w) -> p j w", j=2)[:, :, 0:AWIN],
                    AF.Exp,
                )
                nc.vector.tensor_mul(
                    ets[hh][:, 2 * cp: 2 * cp + 2, :].rearrange("p a b -> p (a b)"),
                    esc[:].rearrange("p a b -> p (a b)"),
                    bmask_sb[:, 2 * cp: 2 * cp + 2, :].rearrange("p a b -> p (a b)"),
                )
        if debug is not None:
            for hh in range(2):
                if ("d_et%d" % (2 * hp + hh)) in debug:
                    nc.sync.dma_start(debug["d_et%d" % (2 * hp + hh)][:], ets[hh][:])
        # P@V with ones column: psum rows 0:64 = O^T, row 64 = sum(E).
        for t in range(NT):
            for hh in range(2):
                h = 2 * hp + hh
                et = ets[hh]
                ot = psum.tile([P, TWIN], f32, name="ot%d" % hh, tag="ot")
                cs = [c for c in range(MC)
                      if min(WIN_START[c] + AWIN, TWIN * (t + 1)) > max(WIN_START[c], TWIN * t)]
                # widest-overlap chunk first so the start=True matmul covers
                # the largest psum range (per-element has_written then only
                # ever accumulates into written elements)
                cs.sort(key=lambda c: max(WIN_START[c], TWIN * t)
                        - min(WIN_START[c] + AWIN, TWIN * (t + 1)))
                for i, c in enumerate(cs):
                    lo = max(WIN_START[c], TWIN * t)
                    hi = min(WIN_START[c] + AWIN, TWIN * (t + 1))
                    # 128-wide weight load (vs 65) keeps FWL enabled; psum
                    # rows 65:128 receive garbage and are never read
                    nc.tensor.matmul(
                        ot[:, lo - TWIN * t: hi - TWIN * t],
                        lhsT=V_flat[:, c, 66 * h: 66 * h + 128],
                        rhs=et[:, c, lo - WIN_START[c]: hi - WIN_START[c]],
                        start=(i == 0),
                        stop=(i == len(cs) - 1),
                    )
                # reciprocal_approx_fast misreads PSUM operands on HW;
                # stage the r-row through SBUF first
                rrow = rpool.tile([1, TWIN], f32, name="rrow", tag="rrow")
                nc.scalar.activation(rrow[:], ot[64:65, :], AF.Copy)
                invrow = rpool.tile([1, TWIN], f32, name="invrow", tag="invrow")
                nc.vector.reciprocal_approx_fast(invrow[:], rrow[:])
                invb = bpool.tile([64, TWIN], f32, name="invb", tag="invb")
                nc.gpsimd.partition_broadcast(invb[:], invrow[:])
                if hh == 0:
                    nc.vector.tensor_mul(
                        OT_sb[0:64, hp, TWIN * t: TWIN * (t + 1)],
                        ot[0:64, :], invb[:],
                    )
                else:
                    st = stpool.tile([64, TWIN], f32r, name="st", tag="st")
                    nc.vector.tensor_mul(st[:], ot[0:64, :], invb[:])
                    nc.sync.dma_start(
                        OT_sb[64:128, hp, TWIN * t: TWIN * (t + 1)], st[:])

    # Split output projection: k-chunks 0..3 (pairs 0..3) accumulate into
    # an SBUF partial while the last attention pairs run, so the PE never
    # idles waiting for the softmax chain; k=4..5 finish at the end.
    KSPLIT = 4
    partial_sb = consts.tile([P, CN, N], f32)

    def emit_proj_part(t):
        for oc in range(CN):
            pps = psum.tile([P, TWIN], f32, name="pps", tag="qkv")
            for k in range(KSPLIT):
                nc.tensor.matmul(
                    pps[:],
                    lhsT=wprojT_sb[:, k, P * oc: P * (oc + 1)],
                    rhs=OT_sb[:, k, TWIN * t: TWIN * (t + 1)],
                    start=(k == 0),
                    stop=(k == KSPLIT - 1),
                )
            nc.vector.tensor_copy(
                partial_sb[:, oc, TWIN * t: TWIN * (t + 1)], pps[:])

    # pair-0 q/k first (starts the PE earliest), then v for heads 0-5,
    # then interleave: attention for pair hp overlaps the qk projection
    # for pair hp+1, the spread-out v chunks for heads 6-11, and the
    # first partial of the output projection.
    emit_qk(0, wqk0)
    emit_v(0, range(MC))
    wq_next = load_wqk(1)
    V1_MS = {0: [0, 1, 2], 1: [3, 4, 5], 2: [6, 7]}
    for hp in range(CN):
        wq_cur = wq_next
        if hp + 1 < CN:
            wq_next = load_wqk(hp + 2) if hp + 2 < CN else None
            emit_qk(hp + 1, wq_cur)
        if hp in V1_MS:
            emit_v(1, V1_MS[hp])
        emit_attention(hp)
        if hp == 3:
            emit_proj_part(0)
        elif hp == 4:
            emit_proj_part(1)

    # ---------------- output projection tail (k = 4..5) ----------------
    add = mybir.AluOpType.add
    with tc.tile_pool(name="outst", bufs=4) as ostpool:
        for oc in range(CN):
            for t in range(NT):
                pps = psum.tile([P, TWIN], f32, name="pps", tag="qkv")
                for k in range(KSPLIT, CN):
                    nc.tensor.matmul(
                        pps[:],
                        lhsT=wprojT_sb[:, k, P * oc: P * (oc + 1)],
                        rhs=OT_sb[:, k, TWIN * t: TWIN * (t + 1)],
                        start=(k == KSPLIT),
                        stop=(k == CN - 1),
                    )
                ost = ostpool.tile([P, TWIN], f32, name="ost", tag="ost")
                # ost = (pps + bias) + partial
                nc.vector.scalar_tensor_tensor(
                    ost[:], pps[:], bias_sb[:, oc: oc + 1],
                    partial_sb[:, oc, TWIN * t: TWIN * (t + 1)],
                    add, add)
                nc.sync.dma_start(
                    outT[P * oc: P * (oc + 1), TWIN * t: TWIN * (t + 1)], ost[:])


def _build(debug_shapes=False):
    global _PROG
    if _PROG is not None:
        return _PROG
    from contextlib import ExitStack

    from concourse import bacc
    import concourse.mybir as mybir
    import concourse.tile as tile

    f32 = mybir.dt.float32
    bf16 = mybir.dt.bfloat16
    f32r = mybir.dt.float32r

    nc = bacc.Bacc("TRN2", target_bir_lowering=False, debug=False,
                   enable_asserts=False)
    xT = nc.dram_tensor("xT", [P, CN * N], bf16, kind="ExternalInput").ap()
    wqkvT = nc.dram_tensor("wqkvT", [P, WQKV_COLS], bf16, kind="ExternalInput").ap()
    wprojT = nc.dram_tensor("wprojT", [P, CN * DIM], f32r, kind="ExternalInput").ap()
    biasT = nc.dram_tensor("biasT", [P, CN], f32, kind="ExternalInput").ap()
    bmask = nc.dram_tensor("bmask", [P, MC, AWIN], bf16, kind="ExternalInput").ap()
    outT = nc.dram_tensor("outT", [DIM, N], f32, kind="ExternalOutput").ap()

    with tile.TileContext(nc) as tc:
        with ExitStack() as ctx:
            _emit(ctx, tc, (xT, wqkvT, wprojT, biasT, bmask, outT))
    nc.compile()
    _PROG = nc
    return nc


def _host_inputs(x, w_qkv, w_proj, b_proj, mask):
    import ml_dtypes

    x = np.asarray(x, dtype=np.float32)
    w_qkv = np.asarray(w_qkv, dtype=np.float32)
    w_proj = np.asarray(w_proj, dtype=np.float32)
    b_proj = np.asarray(b_proj, dtype=np.float32)
    mask = np.asarray(mask, dtype=np.float32)

    wq = w_qkv.copy()
    wq[0:DIM] *= SCALE
    wT = np.ascontiguousarray(wq.T)                          # [768 in, 2304 out]
    # q/k blocks: [p, hp, qk, co, 128] ; v blocks: [p, oh, co, 384]
    wqkv_host = np.empty((P, WQKV_COLS), dtype=np.float32)
    for hp in range(CN):
        for qk in range(2):
            blk = wT[:, DIM * qk + P * hp: DIM * qk + P * hp + P]  # [768, 128]
            blk = blk.reshape(CN, P, P).transpose(1, 0, 2).reshape(P, CN * P)
            base = (2 * hp + qk) * CN * P
            wqkv_host[:, base: base + CN * P] = blk
    for oh in range(2):
        blk = wT[:, 2 * DIM + 384 * oh: 2 * DIM + 384 * (oh + 1)]  # [768, 384]
        blk = blk.reshape(CN, P, 384).transpose(1, 0, 2).reshape(P, CN * 384)
        wqkv_host[:, WV_OFF + oh * CN * 384: WV_OFF + (oh + 1) * CN * 384] = blk
    wqkvT = wqkv_host.astype(ml_dtypes.bfloat16)

    wprojT = np.ascontiguousarray(
        w_proj.T.reshape(CN, P, DIM).transpose(1, 0, 2).reshape(P, CN * DIM))
    biasT = np.ascontiguousarray(b_proj.reshape(CN, P).T)    # [128, 6]

    vis = (mask[0, 0] == 0.0)
    vis_w = vis[np.ix_(PERM, PERM)]
    bm = np.zeros((P, MC, AWIN), dtype=ml_dtypes.bfloat16)
    for c in range(MC):
        s = WIN_START[c]
        bm[:, c, :] = vis_w[c * P:(c + 1) * P, s: s + AWIN]

    in_maps = []
    for b in range(B):
        xTb = np.ascontiguousarray(x[b].T[:, PERM])          # [768, 1024] w-major
        xTb = xTb.reshape(CN, P, N).transpose(1, 0, 2).reshape(P, CN * N)
        in_maps.append({
            "xT": xTb.astype(ml_dtypes.bfloat16),
            "wqkvT": wqkvT,
            "wprojT": wprojT,
            "biasT": biasT,
            "bmask": bm,
        })
    return in_maps


PROFILE = False
LAST_RESULT = None


def kernel(x, w_qkv, w_proj, b_proj, mask):
    global LAST_RESULT
    from concourse.bass_utils import run_bass_kernel_spmd

    nc = _build()
    in_maps = _host_inputs(x, w_qkv, w_proj, b_proj, mask)
    res = run_bass_kernel_spmd(nc, in_maps, core_ids=list(range(B)),
                               trace=PROFILE)
    LAST_RESULT = res
    out = np.empty((B, N, DIM), dtype=np.float32)
    for b in range(B):
        out[b][PERM, :] = np.asarray(res.results[b]["outT"]).T
    return np.ascontiguousarray(out)
